# revision 1
# baseline (speedup 1.0000x reference)
"""Trainium2 Bass kernel for nn_CategoryMultiplier.

out[b, s, :] = inputs[b, s, :] * (emb_table[categories[b, s]] if
               categories[b, s] != 0 else 1.0)

Sharding: pure data parallel over batch. 8 cores x 16 batches each.
Per core: x flat [8192, 512] f32, cats [8192] int32, table [1000, 512] f32.

Device layout: positions are assigned partition-major: partition p holds
positions p*64 .. p*64+63, so input/output DMAs are 64KB-contiguous per
partition (max DMA efficiency). The embedding gather uses one
indirect_dma_start per "column" c (one row gathered per partition).

Padding (category 0 -> multiplier 1.0) is handled by building an internal
copy of the table whose row 0 is all-ones, so no per-element select is
needed: row 0 is only ever gathered by padding positions.
"""

import numpy as np

import concourse.bass as bass
import concourse.bacc as bacc
import concourse.mybir as mybir
import concourse.tile as tile
from concourse.bass_utils import run_bass_kernel_spmd

# Problem shape (hardcoded per harness contract).
B, S, D = 128, 512, 512
VOCAB = 1000
N_CORES = 8
B_LOC = B // N_CORES            # 16 batches per core
N = B_LOC * S                   # 8192 positions per core
P = 128                         # SBUF partitions
C = N // P                      # 64 positions per partition
CHUNK = 8                       # columns (positions/partition) per pipeline chunk
N_CHUNKS = C // CHUNK           # 8

F32 = mybir.dt.float32
I32 = mybir.dt.int32


def _build_nc():
    nc = bacc.Bacc("TRN2", target_bir_lowering=False, debug=False)

    x = nc.dram_tensor("x", [N, D], F32, kind="ExternalInput")
    cats = nc.dram_tensor("cats", [N], I32, kind="ExternalInput")
    table = nc.dram_tensor("table", [VOCAB, D], F32, kind="ExternalInput")
    y = nc.dram_tensor("y", [N, D], F32, kind="ExternalOutput")
    # Internal copy of the table with row 0 replaced by ones.
    table2 = nc.dram_tensor("table2", [VOCAB, D], F32)

    xr = x[:].rearrange("(p c) d -> p (c d)", p=P)     # [128, C*D]
    yr = y[:].rearrange("(p c) d -> p (c d)", p=P)

    with tile.TileContext(nc) as tc:
        with (
            tc.tile_pool(name="const", bufs=1) as const_pool,
            tc.tile_pool(name="io", bufs=3) as io_pool,
            tc.tile_pool(name="gat", bufs=3) as gat_pool,
        ):
            # table2 = table, with row 0 = 1.0 (padding multiplier).
            ones = const_pool.tile([1, D], F32)
            nc.gpsimd.memset(ones[:], 1.0)
            nc.sync.dma_start(out=table2[0:1, :], in_=ones[:])
            nc.sync.dma_start(out=table2[1:VOCAB, :], in_=table[1:VOCAB, :])

            # Category ids, partition-major: cats_t[p, c] = cats[p*C + c].
            cats_t = const_pool.tile([P, C], I32)
            nc.sync.dma_start(out=cats_t[:], in_=cats[:].rearrange("(p c) -> p c", p=P))

            for ch in range(N_CHUNKS):
                lo, hi = ch * CHUNK * D, (ch + 1) * CHUNK * D
                x_t = io_pool.tile([P, CHUNK * D], F32)
                nc.sync.dma_start(out=x_t[:], in_=xr[:, lo:hi])

                g_t = gat_pool.tile([P, CHUNK * D], F32)
                for j in range(CHUNK):
                    c = ch * CHUNK + j
                    nc.gpsimd.indirect_dma_start(
                        out=g_t[:, j * D:(j + 1) * D],
                        out_offset=None,
                        in_=table2[:],
                        in_offset=bass.IndirectOffsetOnAxis(
                            ap=cats_t[:, c:c + 1], axis=0
                        ),
                    )

                nc.vector.tensor_mul(out=g_t[:], in0=g_t[:], in1=x_t[:])
                nc.scalar.dma_start(out=yr[:, lo:hi], in_=g_t[:])

    nc.compile()
    return nc


_NC = None


def _get_nc():
    global _NC
    if _NC is None:
        _NC = _build_nc()
    return _NC


def kernel(inputs, categories, mask_positions=None, emb_table=None, **_):
    """Full (unsharded) inputs in, full output out. mask_positions unused."""
    nc = _get_nc()

    tab = np.ascontiguousarray(emb_table, dtype=np.float32)
    in_maps = []
    for i in range(N_CORES):
        xs = np.ascontiguousarray(
            inputs[i * B_LOC:(i + 1) * B_LOC], dtype=np.float32
        ).reshape(N, D)
        cs = np.ascontiguousarray(
            categories[i * B_LOC:(i + 1) * B_LOC]
        ).reshape(N).astype(np.int32)
        in_maps.append({"x": xs, "cats": cs, "table": tab})

    res = run_bass_kernel_spmd(nc, in_maps, list(range(N_CORES)))
    out = np.concatenate(
        [res.results[i]["y"].reshape(B_LOC, S, D) for i in range(N_CORES)], axis=0
    )
    return out
